# revision 4
# baseline (speedup 1.0000x reference)
"""Trainium2 Bass kernel for nn_MinModel_67388036874289.

Model (per batch b):
    e = one_hot(idx[b], V)                       # [T, V]
    a1 = softmax(causal(Toeplitz(v_weight)))     # [T, T], shared across b
    layer_one = a1 @ e                           # [T, V]
    key_t = layer_one @ W.T                      # [T, V]
    a2 = softmax(causal(e @ key_t.T))            # [T, T]
    logits = a2 @ e                              # [T, V]

Key structure exploited: e is one-hot, so
    a2_pre[t, s] = sum_s' a1[s, s'] * W[idx[t], idx[s']]
which needs only a double gather of W (no dense [T,V]x[V,V] matmuls):
    Wr  = W[idx, :]            (row gather,   indirect DMA)
    WrT = Wr.T                 (PE transpose, bounced through DRAM)
    MT[s',t] = WrT[idx[s'], t] (row gather,   indirect DMA)
    a2_pre = (MT.T @ a1.T)     (TensorE, [T,T]x[T,T], causal-pruned)
and logits = A2 @ e is a scatter-add done as a one-hot matmul:
    logits[t, :] += A2[t, s] at column idx[s]  ==  A2tiles.T @ E

Sharding: data-parallel, one batch per NeuronCore (B == 8 == n_cores).

Self-contained: hardcodes shapes B=8, T=1024, V=2048.
"""

import sys

for _p in ("/opt/trn_rl_repo",):
    if _p not in sys.path:
        sys.path.append(_p)

import numpy as np
import ml_dtypes

import concourse.bass as bass
import concourse.mybir as mybir
import concourse.tile as tile
from concourse.bass_utils import run_bass_kernel_spmd
from concourse.tile import TileContext
from concourse.masks import make_identity
from concourse.vector_clock import ScopedClock

B, T, V = 8, 1024, 2048
P = 128
NT = T // P   # 8 tiles along T
NV = V // P   # 16 tiles along V
F32 = mybir.dt.float32
BF16 = mybir.dt.bfloat16
I32 = mybir.dt.int32

# ---------------------------------------------------------------------------
# Walrus in this environment rejects >1 semaphore wait per instruction
# ("Too many sync wait commands"). Spill extra waits onto same-engine NoOps.
# ---------------------------------------------------------------------------
_MAX_WAITS = 1
_patch_done = False


def _install_tile_patch():
    global _patch_done
    if _patch_done:
        return
    _patch_done = True

    _orig_lower = TileContext._lower_ordered_insts

    def _split(self, ordered):
        for _bb, insts in ordered.items():
            i = 0
            while i < len(insts):
                inst = insts[i]
                si = getattr(inst, "sync_info", None)
                if si is not None and si.on_wait and len(si.on_wait) > _MAX_WAITS:
                    waits = list(si.on_wait)
                    inst.sync_info = mybir.SyncInfo(
                        on_wait=waits[:_MAX_WAITS], on_update=list(si.on_update)
                    )
                    spills = [
                        mybir.InstNoOp(
                            name=self.nc.get_next_instruction_name(),
                            sync_info=mybir.SyncInfo(
                                on_wait=waits[j : j + _MAX_WAITS], on_update=[]
                            ),
                            bass_nofuse=True,
                            engine=inst.engine,
                        )
                        for j in range(_MAX_WAITS, len(waits), _MAX_WAITS)
                    ]
                    insts[i:i] = spills
                    i += len(spills)
                i += 1

    def _patched_lower(self, ordered):
        _split(self, ordered)
        return _orig_lower(self, ordered)

    def _patched_drain_and_barrier(self, tick_clock, wait_clock):
        nc = self.nc
        carrier = nc.sync.nop(nofuse=True)
        wait_clock.add_sem_waits(
            carrier.ins, ScopedClock({None: tick_clock.global_clock})
        )
        si = carrier.ins.sync_info
        if si is not None and len(si.on_wait) > _MAX_WAITS:
            waits = list(si.on_wait)
            carrier.ins.sync_info = mybir.SyncInfo(
                on_wait=waits[:_MAX_WAITS], on_update=list(si.on_update)
            )
            for i in range(_MAX_WAITS, len(waits), _MAX_WAITS):
                extra = nc.sync.nop(nofuse=True)
                extra.ins.sync_info = mybir.SyncInfo(
                    on_wait=waits[i : i + _MAX_WAITS], on_update=[]
                )
        nc.sync.drain()
        nc.all_engine_barrier()
        assert self.sems is not None
        popped = nc._tile_sem_poison_stack.pop()
        assert popped is self._sem_poison
        nc.clear_and_free_semaphores(list(self.sems.allocated().values()))
        nc.all_engine_barrier()

    TileContext._lower_ordered_insts = _patched_lower
    TileContext._drain_and_barrier = _patched_drain_and_barrier


# ---------------------------------------------------------------------------
# Kernel build
# ---------------------------------------------------------------------------
_nc_cache = None


def _build():
    global _nc_cache
    if _nc_cache is not None:
        return _nc_cache
    _install_tile_patch()

    nc = bass.Bass(num_devices=B)
    idx_d = nc.declare_dram_parameter("idx", [NT, P], I32, isOutput=False)
    a1raw_d = nc.declare_dram_parameter("a1raw", [T, T], BF16, isOutput=False)
    w_d = nc.declare_dram_parameter("W", [V, V], F32, isOutput=False)
    out_d = nc.declare_dram_parameter("out", [T, V], F32, isOutput=True)

    wrt_dram = nc.dram_tensor("wrt_scratch", [V, T], BF16)

    with TileContext(nc) as tc:
        with (
            tc.tile_pool(name="const", bufs=1) as cpool,
            tc.tile_pool(name="persist", bufs=1) as pers,
            tc.tile_pool(name="work", bufs=2) as work,
            tc.tile_pool(name="stats", bufs=4) as stats,
            tc.tile_pool(name="pst", bufs=4, space="PSUM") as pst,
            tc.tile_pool(name="psmm", bufs=4, space="PSUM") as psmm,
        ):
            # ---- constants ----
            ident_f = cpool.tile([P, P], F32)
            make_identity(nc, ident_f[:])
            ident_b = cpool.tile([P, P], BF16)
            nc.vector.tensor_copy(out=ident_b[:], in_=ident_f[:])

            idx_sb = cpool.tile([P, NT], I32)
            nc.sync.dma_start(out=idx_sb[:], in_=idx_d.rearrange("a p -> p a"))

            iota = cpool.tile([P, V], I32)
            nc.gpsimd.iota(
                out=iota[:], pattern=[[1, V]], base=0, channel_multiplier=0
            )

            # ---- persistent arrays ----
            wrt = [pers.tile([P, T], BF16, name=f"wrt{vj}", tag=f"wrt{vj}") for vj in range(NV)]
            mt = [pers.tile([P, T], BF16, name=f"mt{si}", tag=f"mt{si}") for si in range(NT)]
            a1t = [pers.tile([P, T], BF16, name=f"a1t{si}", tag=f"a1t{si}") for si in range(NT)]
            E = [pers.tile([P, V], BF16, name=f"E{si}", tag=f"E{si}") for si in range(NT)]
            a2t = [pers.tile([P, T], BF16, name=f"a2t{si}", tag=f"a2t{si}") for si in range(NT)]

            # ---- phase A: gather W rows, transpose, bounce to DRAM ----
            with tc.tile_pool(name="wrpool", bufs=3) as wrpool:
                for ti in range(NT):
                    wr = wrpool.tile([P, V], F32, tag="wr")
                    nc.gpsimd.indirect_dma_start(
                        out=wr[:],
                        out_offset=None,
                        in_=w_d[:, :],
                        in_offset=bass.IndirectOffsetOnAxis(
                            ap=idx_sb[:, ti : ti + 1], axis=0
                        ),
                    )
                    for vj in range(NV):
                        pt = pst.tile([P, P], F32, space="PSUM", tag="pt")
                        nc.tensor.transpose(
                            out=pt[:],
                            in_=wr[:, vj * P : (vj + 1) * P],
                            identity=ident_f[:],
                        )
                        nc.any.tensor_copy(
                            out=wrt[vj][:, ti * P : (ti + 1) * P], in_=pt[:]
                        )
                for vj in range(NV):
                    nc.sync.dma_start(
                        out=wrt_dram[vj * P : (vj + 1) * P, :], in_=wrt[vj][:]
                    )

            # ---- phase A2: a1 softmax + transpose (independent track) ----
            for ti in range(NT):
                raw = work.tile([P, T], BF16, tag="a1raw")
                nc.sync.dma_start(
                    out=raw[:], in_=a1raw_d[ti * P : (ti + 1) * P, :]
                )
                e1 = work.tile([P, T], F32, tag="e1")
                s1 = stats.tile([P, 1], F32, tag="s1")
                nc.scalar.activation(
                    out=e1[:],
                    in_=raw[:],
                    func=mybir.ActivationFunctionType.Exp,
                    accum_out=s1[:],
                )
                r1 = stats.tile([P, 1], F32, tag="r1")
                nc.vector.reciprocal(out=r1[:], in_=s1[:])
                a1n = work.tile([P, T], BF16, tag="a1n")
                nc.vector.tensor_scalar_mul(a1n[:], e1[:], r1[:, :1])
                for sj in range(NT):
                    pt = pst.tile([P, P], BF16, space="PSUM", tag="pt")
                    nc.tensor.transpose(
                        out=pt[:],
                        in_=a1n[:, sj * P : (sj + 1) * P],
                        identity=ident_b[:],
                    )
                    nc.any.tensor_copy(
                        out=a1t[sj][:, ti * P : (ti + 1) * P], in_=pt[:]
                    )

            # ---- phase A3: one-hot E tiles ----
            for si in range(NT):
                nc.vector.tensor_tensor(
                    out=E[si][:],
                    in0=iota[:],
                    in1=idx_sb[:, si : si + 1].to_broadcast([P, V]),
                    op=mybir.AluOpType.is_equal,
                )

            # ---- phase B: second gather ----
            for si in range(NT):
                nc.gpsimd.indirect_dma_start(
                    out=mt[si][:],
                    out_offset=None,
                    in_=wrt_dram[:, :],
                    in_offset=bass.IndirectOffsetOnAxis(
                        ap=idx_sb[:, si : si + 1], axis=0
                    ),
                )

            # ---- phase C: a2 matmul + causal softmax + transpose ----
            for ti in range(NT):
                nsj = 1 if ti < 4 else 2
                width = nsj * 512
                ea2 = work.tile([P, T], F32, tag="ea2")
                for sj in range(nsj):
                    ps_a2 = psmm.tile([P, 512], F32, space="PSUM", tag="mm")
                    klist = list(range(0, min(ti, 4 * sj + 3) + 1))
                    for n, si in enumerate(klist):
                        nc.tensor.matmul(
                            out=ps_a2[:],
                            lhsT=mt[si][:, ti * P : (ti + 1) * P],
                            rhs=a1t[si][:, sj * 512 : (sj + 1) * 512],
                            start=(n == 0),
                            stop=(n == len(klist) - 1),
                        )
                    nc.scalar.activation(
                        out=ea2[:, sj * 512 : (sj + 1) * 512],
                        in_=ps_a2[:],
                        func=mybir.ActivationFunctionType.Exp,
                    )
                # causal mask: keep where 128*ti + p - s >= 0, else 0
                nc.gpsimd.affine_select(
                    out=ea2[:, :width],
                    in_=ea2[:, :width],
                    compare_op=mybir.AluOpType.is_ge,
                    fill=0.0,
                    base=128 * ti,
                    pattern=[[-1, width]],
                    channel_multiplier=1,
                )
                s2 = stats.tile([P, 1], F32, tag="s2")
                nc.vector.reduce_sum(
                    out=s2[:], in_=ea2[:, :width], axis=mybir.AxisListType.X
                )
                r2 = stats.tile([P, 1], F32, tag="r2")
                nc.vector.reciprocal(out=r2[:], in_=s2[:])
                A2 = work.tile([P, T], BF16, tag="A2")
                nc.vector.tensor_scalar_mul(A2[:, :width], ea2[:, :width], r2[:, :1])
                for si in range(ti + 1):
                    pt = pst.tile([P, P], BF16, space="PSUM", tag="pt")
                    nc.tensor.transpose(
                        out=pt[:],
                        in_=A2[:, si * P : (si + 1) * P],
                        identity=ident_b[:],
                    )
                    nc.any.tensor_copy(
                        out=a2t[si][:, ti * P : (ti + 1) * P], in_=pt[:]
                    )

            # ---- phase D: logits = A2 @ E (one-hot scatter matmul) ----
            for ti in range(NT):
                orow = work.tile([P, V], F32, tag="orow")
                for vj in range(4):
                    ps3 = psmm.tile([P, 512], F32, space="PSUM", tag="mm")
                    for si in range(ti + 1):
                        nc.tensor.matmul(
                            out=ps3[:],
                            lhsT=a2t[si][:, ti * P : (ti + 1) * P],
                            rhs=E[si][:, vj * 512 : (vj + 1) * 512],
                            start=(si == 0),
                            stop=(si == ti),
                        )
                    nc.any.tensor_copy(
                        out=orow[:, vj * 512 : (vj + 1) * 512], in_=ps3[:]
                    )
                nc.sync.dma_start(
                    out=out_d[ti * P : (ti + 1) * P, :], in_=orow[:]
                )

    _nc_cache = nc
    return nc


def _host_a1raw(v_weight: np.ndarray) -> np.ndarray:
    """Toeplitz layout of v_weight with -1e30 on the masked (upper) part.

    a1raw[q, k] = v_weight[q - k] for k <= q, else -1e30. Pure data
    rearrangement of the v_weight input (the gather the model's
    v_weight[rel] indexing performs); all arithmetic stays on device.
    """
    v = np.asarray(v_weight, dtype=np.float32).reshape(-1)
    q = np.arange(T)
    relmat = np.clip(q[:, None] - q[None, :], 0, None)
    raw = v[relmat]
    raw[q[:, None] < q[None, :]] = -1e30
    return raw.astype(ml_dtypes.bfloat16)


def kernel(idx, v_weight, W):
    nc = _build()
    idx_np = np.asarray(idx).astype(np.int32)
    w_np = np.ascontiguousarray(np.asarray(W, dtype=np.float32))
    a1raw = _host_a1raw(v_weight)

    in_maps = [
        {
            "idx": np.ascontiguousarray(idx_np[b].reshape(NT, P)),
            "a1raw": a1raw,
            "W": w_np,
        }
        for b in range(B)
    ]
    res = run_bass_kernel_spmd(nc, in_maps, list(range(B)))
    return np.stack([np.asarray(res.results[b]["out"]) for b in range(B)], axis=0)
